# revision 1
# baseline (speedup 1.0000x reference)
"""Trainium2 Bass kernel for nn_Attention_Sep (sparse attention, B=16 N=1025 C=768 H=12 nb=4).

Data-parallel over batch (2 per core, 8 cores). Per core, fp32r matmuls in a
transposed [feature, token] layout:
  A) Q^T/K^T feature-tiles = W_qkv.T @ x^T; V natural (x^T as lhsT) stored in an
     augmented head-pair group layout [A(64)|ones|pad31|B(64)] (160 cols/group)
     so the PV matmul later produces softmax denominators for free.
  B) cls attention over all 1025 tokens: block-diagonal q0 lhsT -> [12, tok]
     logits, row softmax (exp+accum on ACT), PE-transposed weights, V contraction,
     + residual -> A^T column 0.
  C) recompute k,v of the updated cls token (row matmuls + tiny transposes).
  D) 4 branches x 6 head-pairs: S^T row-pair-packed (2 heads concurrently in
     different PSUM banks), exp with folded 1/8 scale, PV with denominators at
     rows 64 (head A, group[0:65]) / 32 (head B, group[32:160]); reciprocal +
     DMA partition-broadcast + DVE multiply -> normalized A^T in place.
  E) out = A^T.T @ W_proj + b_proj -> DRAM natural layout.
"""
import sys, types
import numpy as np


def _ensure_ntff_hook():
    try:
        import antenv
        if "antenv.axon_hooks" in sys.modules:
            return
        from trn_agent_boot.trn_boot import _ntff_profile_via_ctypes
        mod = types.ModuleType("antenv.axon_hooks")
        mod._hook = None
        mod.set_axon_ntff_profile_hook = lambda h: setattr(mod, "_hook", h)
        mod.get_axon_ntff_profile_hook = lambda: mod._hook
        sys.modules["antenv.axon_hooks"] = mod
        antenv.axon_hooks = mod
        mod.set_axon_ntff_profile_hook(_ntff_profile_via_ctypes('/opt/axon/libaxon_pjrt.so'))
    except Exception:
        pass


_NC_CACHE = {}


def build_program():
    if "nc" in _NC_CACHE:
        return _NC_CACHE["nc"]
    import concourse.bass as bass
    import concourse.mybir as mybir
    import concourse.tile as tile
    from concourse import bacc
    from concourse.masks import make_identity

    F32, F32R = mybir.dt.float32, mybir.dt.float32r
    AF = mybir.ActivationFunctionType
    SCALE = 0.125
    G = 160

    nc = bacc.Bacc("TRN2", target_bir_lowering=False, debug=False)
    xt_d = [nc.dram_tensor(f"xt{b}", [768, 1025], F32R, kind="ExternalInput") for b in range(2)]
    wq_d = nc.dram_tensor("wqkv", [768, 2304], F32R, kind="ExternalInput")
    wp_d = nc.dram_tensor("wproj", [768, 768], F32R, kind="ExternalInput")
    bias_d = nc.dram_tensor("bias", [1, 768], F32, kind="ExternalInput")
    tpl_d = nc.dram_tensor("tpl", [128, 160], F32R, kind="ExternalInput")
    out_d = [nc.dram_tensor(f"out{b}", [1025, 768], F32, kind="ExternalOutput") for b in range(2)]

    with tile.TileContext(nc) as tc:
        with (
            tc.tile_pool(name="big", bufs=1) as big,
            tc.tile_pool(name="qwring", bufs=2) as qwring,
            tc.tile_pool(name="bigring", bufs=1) as bigring,
            tc.tile_pool(name="aug", bufs=2) as augring,
            tc.tile_pool(name="es", bufs=1) as es_pool,
            tc.tile_pool(name="nm", bufs=2) as nm_pool,
            tc.tile_pool(name="st", bufs=2) as st_pool,
            tc.tile_pool(name="row", bufs=1) as row_pool,
            tc.tile_pool(name="ps1", bufs=1, space="PSUM") as ps1,
            tc.tile_pool(name="pscls", bufs=2, space="PSUM") as pscls,
            tc.tile_pool(name="pso", bufs=2, space="PSUM") as pso,
            tc.tile_pool(name="psmm", bufs=2, space="PSUM") as psmm,
        ):
            # persistent
            vw = big.tile([128, 6, 768], F32R, tag="vw")  # W_qkv v-cols 1536:2304
            for k in range(6):
                nc.sync.dma_start(vw[:, k], wq_d.ap()[k * 128:(k + 1) * 128, 1536:2304])
            bias1 = big.tile([1, 768], F32, tag="bias1")
            nc.sync.dma_start(bias1[:], bias_d.ap())
            biasb = big.tile([128, 768], F32, tag="biasb")
            nc.sync.dma_start(biasb[:], bass.AP(bias1.tensor, bias1.offset, [[768, 1], [0, 128], [1, 768]]))
            ident = big.tile([128, 128], F32, tag="ident")
            make_identity(nc, ident[:])

            xt = big.tile([128, 6, 1025], F32R, tag="xt")
            qkt = big.tile([128, 12, 1024], F32R, tag="qkt")
            vstore = big.tile([128, 8, 6, G], F32R, tag="vstore")
            at = big.tile([128, 6, 1025], F32R, tag="at")
            x0col = big.tile([128, 6], F32R, tag="x0col")
            kct = [big.tile([128, 6], F32R, tag=f"kct{b}", name=f"kct{b}") for b in range(2)]
            q0blk = big.tile([128, 6, 12], F32R, tag="q0blk")
            q0cp = big.tile([128, 6], F32R, tag="q0cp")
            k0cp = big.tile([128, 6], F32R, tag="k0cp")
            w_cls = big.tile([12, 1025], F32, tag="wcls")
            wt_cls = big.tile([128, 9, 12], F32R, tag="wtcls")
            part = big.tile([12, 4], F32, tag="part")
            den_c = big.tile([12, 2], F32, tag="denc")
            clsnew = big.tile([128, 6], F32R, tag="clsnew")

            for t in range(8):
                for g in range(6):
                    nc.sync.dma_start(vstore[:, t, g], tpl_d.ap())

            VS_P = 8 * 6 * G

            def run_batch(b):
                # ---- stage A ----
                for k in range(6):
                    nc.sync.dma_start(xt[:, k], xt_d[b].ap()[k * 128:(k + 1) * 128, :])
                for k in range(6):
                    nc.vector.tensor_copy(x0col[:, k:k + 1], xt[:, k, 0:1])
                for m in range(12):
                    wm = qwring.tile([128, 6, 128], F32R, tag="qw")
                    for k in range(6):
                        nc.sync.dma_start(wm[:, k], wq_d.ap()[k * 128:(k + 1) * 128, m * 128:(m + 1) * 128])
                    for ci in range(2):
                        qk_ps = psmm.tile([128, 512], F32, tag="mm")
                        for k in range(6):
                            nc.tensor.matmul(qk_ps[:], wm[:, k], xt[:, k, 1 + ci * 512: 1 + (ci + 1) * 512],
                                             start=(k == 0), stop=(k == 5))
                        nc.vector.tensor_copy(qkt[:, m, ci * 512:(ci + 1) * 512], qk_ps[:])
                    if m < 6:
                        # q0 column for this feature tile (cls query)
                        q0_ps = pscls.tile([128, 512], F32, tag="cls")
                        for k in range(6):
                            nc.tensor.matmul(q0_ps[0:128, 0:1], wm[:, k].bitcast(F32), x0col[:, k:k + 1].bitcast(F32),
                                             start=(k == 0), stop=(k == 5))
                        nc.vector.tensor_copy(q0cp[:, m:m + 1], q0_ps[0:128, 0:1])
                for t in range(8):
                    for ci, (c0, cw, g0) in enumerate(((0, 512, 0), (512, 256, 4))):
                        v_ps = psmm.tile([128, 512], F32, tag="mm")
                        for k in range(6):
                            nc.tensor.matmul(v_ps[:, 0:cw], xt[:, k, 1 + t * 128: 1 + (t + 1) * 128],
                                             vw[:, k, c0:c0 + cw], start=(k == 0), stop=(k == 5))
                        dst = bass.AP(vstore.tensor, vstore.offset + t * 6 * G + g0 * G,
                                      [[VS_P, 128], [G, cw // 128], [96, 2], [1, 64]])
                        nc.vector.tensor_copy(dst, v_ps[:, 0:cw].rearrange("p (g h d) -> p g h d", h=2, d=64))

                # ---- k0 | v0 row of the original cls token ----
                k0v0 = row_pool.tile([1, 1536], F32, tag="rowbuf")
                for (dst0, cw, src_kind, s0) in ((0, 512, "stream", 768), (512, 256, "stream", 1280),
                                                 (768, 512, "vw", 0), (1280, 256, "vw", 512)):
                    if src_kind == "stream":
                        wr = bigring.tile([128, 6, 512], F32R, tag="bigr")
                        for k in range(6):
                            nc.sync.dma_start(wr[:, k, 0:cw], wq_d.ap()[k * 128:(k + 1) * 128, s0:s0 + cw])
                    r_ps = pscls.tile([128, 512], F32, tag="cls")
                    for k in range(6):
                        rhs = wr[:, k, 0:cw] if src_kind == "stream" else vw[:, k, s0:s0 + cw]
                        nc.tensor.matmul(r_ps[0:1, 0:cw], x0col[:, k:k + 1], rhs,
                                         start=(k == 0), stop=(k == 5))
                    nc.vector.tensor_copy(k0v0[0:1, dst0:dst0 + cw], r_ps[0:1, 0:cw])
                for k in range(6):
                    tp = pscls.tile([128, 512], F32, tag="cls")
                    nc.tensor.transpose(tp[0:128, 0:1], k0v0[0:1, k * 128:(k + 1) * 128], ident[0:1, 0:1])
                    nc.vector.tensor_copy(k0cp[:, k:k + 1], tp[0:128, 0:1])
                v0a = augring.tile([1, 6, G], F32R, tag="aug")
                for g in range(6):
                    nc.sync.dma_start(v0a[:, g], tpl_d.ap()[0:1, :])
                for (c0, ng, g0) in ((768, 4, 0), (1280, 2, 4)):
                    dst = bass.AP(v0a.tensor, v0a.offset + g0 * G, [[6 * G, 1], [G, ng], [96, 2], [1, 64]])
                    nc.vector.tensor_copy(dst, k0v0[0:1, c0:c0 + ng * 128].rearrange("p (g h d) -> p g h d", h=2, d=64))

                # ---- stage B: cls attention ----
                nc.sync.dma_start(q0blk.rearrange("p a c -> p (a c)")[:, 0:64], tpl_d.ap()[:, 0:64])
                nc.sync.dma_start(q0blk.rearrange("p a c -> p (a c)")[:, 64:72], tpl_d.ap()[:, 65:73])
                for k in range(6):
                    nc.vector.tensor_copy(q0blk[0:64, k, 2 * k:2 * k + 1], q0cp[0:64, k:k + 1])
                    nc.vector.tensor_copy(q0blk[64:128, k, 2 * k + 1:2 * k + 2], q0cp[64:128, k:k + 1])
                for ci in range(2):
                    cl_ps = pscls.tile([128, 512], F32, tag="cls")
                    for k in range(6):
                        nc.tensor.matmul(cl_ps[0:12, :], q0blk[:, k], qkt[:, 6 + k, ci * 512:(ci + 1) * 512],
                                         start=(k == 0), stop=(k == 5))
                    nc.scalar.activation(w_cls[:, 1 + ci * 512: 1 + (ci + 1) * 512], cl_ps[0:12, :],
                                         AF.Exp, scale=SCALE, accum_out=part[:, ci:ci + 1])
                cl0 = pscls.tile([128, 512], F32, tag="cls")
                for k in range(6):
                    nc.tensor.matmul(cl0[0:12, 0:1], q0blk[:, k].bitcast(F32), k0cp[:, k:k + 1].bitcast(F32),
                                     start=(k == 0), stop=(k == 5))
                nc.scalar.activation(w_cls[:, 0:1], cl0[0:12, 0:1], AF.Exp, scale=SCALE,
                                     accum_out=part[:, 2:3])
                nc.vector.tensor_add(den_c[:, 0:1], part[:, 0:1], part[:, 1:2])
                nc.vector.tensor_add(den_c[:, 0:1], den_c[:, 0:1], part[:, 2:3])
                nc.vector.reciprocal(den_c[:, 1:2], den_c[:, 0:1])
                nc.vector.tensor_scalar_mul(w_cls[:], w_cls[:], den_c[:, 1:2])
                t0 = pscls.tile([128, 512], F32, tag="cls")
                nc.tensor.transpose(t0[0:1, 0:12], w_cls[:, 0:1], ident[0:12, 0:12])
                nc.vector.tensor_copy(wt_cls[0:1, 0, :], t0[0:1, 0:12])
                for t in range(8):
                    tw = pscls.tile([128, 512], F32, tag="cls")
                    nc.tensor.transpose(tw[0:128, 0:12], w_cls[:, 1 + t * 128: 1 + (t + 1) * 128],
                                        ident[0:12, 0:12])
                    nc.vector.tensor_copy(wt_cls[:, 1 + t, :], tw[0:128, 0:12])
                for fi in range(6):
                    co_a = pscls.tile([128, 512], F32, tag="cls")
                    co_b = pscls.tile([128, 512], F32, tag="cls")
                    lA0 = bass.AP(v0a.tensor, v0a.offset + fi * G, [[6 * G, 1], [1, 64]])
                    nc.tensor.matmul(co_a[0:64, 0:12], lA0, wt_cls[0:1, 0, :], start=True, stop=False)
                    lB0 = bass.AP(v0a.tensor, v0a.offset + fi * G + 32, [[6 * G, 1], [1, 128]])
                    nc.tensor.matmul(co_b[0:128, 0:12], lB0, wt_cls[0:1, 0, :], start=True, stop=False)
                    for t in range(8):
                        lA = bass.AP(vstore.tensor, vstore.offset + t * 6 * G + fi * G,
                                     [[VS_P, 128], [1, 64]])
                        nc.tensor.matmul(co_a[0:64, 0:12], lA, wt_cls[:, 1 + t, :],
                                         start=False, stop=(t == 7))
                        lB = bass.AP(vstore.tensor, vstore.offset + t * 6 * G + fi * G + 32,
                                     [[VS_P, 128], [1, 128]])
                        nc.tensor.matmul(co_b[0:128, 0:12], lB, wt_cls[:, 1 + t, :],
                                         start=False, stop=(t == 7))
                    nc.vector.tensor_add(at[0:64, fi, 0:1], co_a[0:64, 2 * fi:2 * fi + 1], x0col[0:64, fi:fi + 1])
                    nc.vector.tensor_add(at[64:128, fi, 0:1], co_b[64:128, 2 * fi + 1:2 * fi + 2], x0col[64:128, fi:fi + 1])

                # ---- stage C: updated cls k, v ----
                for k in range(6):
                    nc.vector.tensor_copy(clsnew[:, k:k + 1], at[:, k, 0:1])
                kvc = row_pool.tile([1, 1536], F32, tag="rowbuf")
                for (dst0, s0) in ((0, 768), (512, 1280), (1024, 1792)):
                    wr = bigring.tile([128, 6, 512], F32R, tag="bigr")
                    for k in range(6):
                        nc.sync.dma_start(wr[:, k], wq_d.ap()[k * 128:(k + 1) * 128, s0:s0 + 512])
                    kv_ps = pscls.tile([128, 512], F32, tag="cls")
                    for k in range(6):
                        nc.tensor.matmul(kv_ps[0:1, :], clsnew[:, k:k + 1], wr[:, k],
                                         start=(k == 0), stop=(k == 5))
                    nc.vector.tensor_copy(kvc[0:1, dst0:dst0 + 512], kv_ps[0:1, :])
                for k in range(6):
                    tk = pscls.tile([128, 512], F32, tag="cls")
                    nc.tensor.transpose(tk[0:128, 0:1], kvc[0:1, k * 128:(k + 1) * 128], ident[0:1, 0:1])
                    nc.vector.tensor_copy(kct[b][:, k:k + 1], tk[0:128, 0:1])
                vca = augring.tile([1, 6, G], F32R, tag="aug")
                for g in range(6):
                    nc.sync.dma_start(vca[:, g], tpl_d.ap()[0:1, :])
                for (c0, ng, g0) in ((768, 4, 0), (1280, 2, 4)):
                    dst = bass.AP(vca.tensor, vca.offset + g0 * G, [[6 * G, 1], [G, ng], [96, 2], [1, 64]])
                    nc.vector.tensor_copy(dst, kvc[0:1, c0:c0 + ng * 128].rearrange("p (g h d) -> p g h d", h=2, d=64))

                # ---- stage D: branch attention ----
                for br in range(4):
                    for fi in range(6):
                        qsl = slice(br * 256, (br + 1) * 256)
                        ps_sa = ps1.tile([128, 512], F32, tag="sa")
                        ps_sb = ps1.tile([128, 512], F32, tag="sb")
                        for half in range(2):
                            ksl = slice(br * 256 + half * 128, br * 256 + (half + 1) * 128)
                            nc.tensor.matmul(ps_sa[:, half * 256:(half + 1) * 256],
                                             qkt[0:64, 6 + fi, ksl], qkt[0:64, fi, qsl],
                                             start=True, stop=True)
                            nc.tensor.matmul(ps_sb[:, half * 256:(half + 1) * 256],
                                             qkt[64:128, 6 + fi, ksl], qkt[64:128, fi, qsl],
                                             start=True, stop=True)
                        ps_ca = pscls.tile([128, 512], F32, tag="cls")
                        ps_cb = pscls.tile([128, 512], F32, tag="cls")
                        nc.tensor.matmul(ps_ca[0:1, 0:256], kct[b][0:64, fi:fi + 1], qkt[0:64, fi, qsl],
                                         start=True, stop=True)
                        nc.tensor.matmul(ps_cb[0:1, 0:256], kct[b][64:128, fi:fi + 1], qkt[64:128, fi, qsl],
                                         start=True, stop=True)
                        esa = es_pool.tile([128, 512], F32R, tag="esa")
                        esb = es_pool.tile([128, 512], F32R, tag="esb")
                        esc = es_pool.tile([1, 512], F32R, tag="esc")
                        nc.scalar.activation(esa[:], ps_sa[:], AF.Exp, scale=SCALE)
                        nc.scalar.activation(esb[:], ps_sb[:], AF.Exp, scale=SCALE)
                        nc.scalar.activation(esc[0:1, 0:256], ps_ca[0:1, 0:256], AF.Exp, scale=SCALE)
                        nc.scalar.activation(esc[0:1, 256:512], ps_cb[0:1, 0:256], AF.Exp, scale=SCALE)
                        ps_oa = pso.tile([128, 256], F32, tag="o")
                        ps_ob = psmm.tile([128, 256], F32, tag="mm")
                        lhsA0 = bass.AP(vca.tensor, vca.offset + fi * G, [[6 * G, 1], [1, 65]])
                        nc.tensor.matmul(ps_oa[0:65, :], lhsA0, esc[0:1, 0:256], start=True, stop=False)
                        lhsB0 = bass.AP(vca.tensor, vca.offset + fi * G + 32, [[6 * G, 1], [1, 128]])
                        nc.tensor.matmul(ps_ob[0:128, :], lhsB0, esc[0:1, 256:512], start=True, stop=False)
                        for half in range(2):
                            tt = 2 * br + half
                            lhsA = bass.AP(vstore.tensor, vstore.offset + tt * 6 * G + fi * G,
                                           [[VS_P, 128], [1, 65]])
                            nc.tensor.matmul(ps_oa[0:65, :], lhsA, esa[:, half * 256:(half + 1) * 256],
                                             start=False, stop=(half == 1))
                            lhsB = bass.AP(vstore.tensor, vstore.offset + tt * 6 * G + fi * G + 32,
                                           [[VS_P, 128], [1, 128]])
                            nc.tensor.matmul(ps_ob[0:128, :], lhsB, esb[:, half * 256:(half + 1) * 256],
                                             start=False, stop=(half == 1))
                        # stage denominator rows to SBUF, DMA-broadcast, then wide reciprocal
                        ra = nm_pool.tile([128, 256], F32, tag="ra")
                        nc.vector.tensor_copy(ra[64:65, :], ps_oa[64:65, :])
                        nc.vector.tensor_copy(ra[32:33, :], ps_ob[32:33, :])
                        rb = nm_pool.tile([128, 256], F32, tag="rb")
                        nc.sync.dma_start(rb[0:64, :], bass.AP(ra.tensor, ra.offset + 64 * 256,
                                                               [[256, 1], [0, 64], [1, 256]]))
                        nc.sync.dma_start(rb[64:128, :], bass.AP(ra.tensor, ra.offset + 32 * 256,
                                                                 [[256, 1], [0, 64], [1, 256]]))
                        nc.vector.reciprocal(rb[:, :], rb[:, :])
                        csl = slice(1 + br * 256, 1 + (br + 1) * 256)
                        nc.vector.tensor_mul(at[0:64, fi, csl], ps_oa[0:64, :], rb[0:64, :])
                        nc.vector.tensor_mul(at[64:128, fi, csl], ps_ob[64:128, :], rb[64:128, :])

                # ---- stage E: projection ----
                for (c0, cw) in ((0, 512), (512, 256)):
                    wpc = bigring.tile([128, 6, 512], F32R, tag="bigr")
                    for k in range(6):
                        nc.sync.dma_start(wpc[:, k, 0:cw], wp_d.ap()[k * 128:(k + 1) * 128, c0:c0 + cw])
                    for mt in range(9):
                        m0, mw = (mt * 128, 128) if mt < 8 else (1024, 1)
                        pr = psmm.tile([128, 512], F32, tag="mm")
                        for k in range(6):
                            nc.tensor.matmul(pr[0:mw, 0:cw], at[:, k, m0:m0 + mw], wpc[:, k, 0:cw],
                                             start=(k == 0), stop=(k == 5))
                        stg = st_pool.tile([128, 512], F32, tag="stg")
                        nc.vector.tensor_add(stg[0:mw, 0:cw], pr[0:mw, 0:cw], biasb[0:mw, c0:c0 + cw])
                        nc.sync.dma_start(out_d[b].ap()[m0:m0 + mw, c0:c0 + cw], stg[0:mw, 0:cw])

            run_batch(0)
            run_batch(1)

    nc.compile()
    _NC_CACHE["nc"] = nc
    return nc


def kernel(x, W_qkv, W_proj, b_proj):
    _ensure_ntff_hook()
    from concourse import bass_utils
    x = np.asarray(x, dtype=np.float32)
    W_qkv = np.asarray(W_qkv, dtype=np.float32)
    W_proj = np.asarray(W_proj, dtype=np.float32)
    b_proj = np.asarray(b_proj, dtype=np.float32)

    nc = build_program()
    xt = np.ascontiguousarray(np.transpose(x, (0, 2, 1)))
    tpl = np.zeros((128, 160), np.float32)
    tpl[:, 64] = 1.0
    bias = np.ascontiguousarray(b_proj.reshape(1, 768))
    in_maps = [{"xt0": xt[2 * c], "xt1": xt[2 * c + 1],
                "wqkv": W_qkv, "wproj": W_proj, "bias": bias, "tpl": tpl}
               for c in range(8)]
    res = bass_utils.run_bass_kernel_spmd(nc, in_maps, list(range(8)))
    out = np.empty((16, 1025, 768), np.float32)
    for c in range(8):
        out[2 * c] = res.results[c]["out0"]
        out[2 * c + 1] = res.results[c]["out1"]
    return out



# revision 19
# speedup vs baseline: 1.2240x; 1.2240x over previous
"""Trainium2 Bass kernel for nn_Attention_Sep (sparse attention, B=16 N=1025 C=768 H=12 nb=4).

Data-parallel over batch (2 per core, 8 cores), bf16 matmuls with fp32 PSUM.
Per core, transposed [feature, token] layout:
  A) Q^T/K^T feature-tiles = W_qkv.T @ x^T; V natural (x^T as lhsT) stored in an
     augmented head-pair group layout [A(64)|ones|pad31|B(64)] (160 cols/group)
     so the PV matmul later produces softmax denominators for free. Full W_qkv /
     W_proj stay resident in SBUF (bf16) across both batches.
  B) cls attention over all 1025 tokens: block-diagonal q0 lhsT -> [12, tok]
     logits, row softmax (exp+accum on ACT), PE-transposed weights, V contraction
     per (fi): a-chain rows 0:64 + b-chain rows 64:128, + residual -> A^T col 0.
  C) recompute k,v of the updated cls token (row matmuls + tiny transposes).
  D) 4 branches x 6 head-pairs: S^T per 128-key half (2 heads concurrently via
     row groups, separate PSUM banks), one merged exp per half into an
     (a0|b0|a1|b1) bf16 buffer, PV with denominators at psum rows 64 (head A) /
     32 (head B); reciprocal straight from PSUM + DMA partition-broadcast +
     DVE multiply -> A^T.
  E) out = A^T.T @ W_proj + b_proj (bias via zero-padded full-K matmul).

HW constraint honored throughout: matmuls in different PE row-groups run
CONCURRENTLY and must never drain into the same PSUM bank (hangs the device).
So every K=1 contribution (cls rows, bias) is expressed as a full-K matmul
whose lhsT/rhs rows 1:128 are zeros, keeping row-group usage uniform.
Both batches' tiles come from bufs=2 rings so the Tile scheduler pipelines
batch 1's stage A under batch 0's serial cls chain and attention.
"""
import os, sys, types
import numpy as np


def _ensure_ntff_hook():
    try:
        import antenv
        if "antenv.axon_hooks" in sys.modules:
            return
        from trn_agent_boot.trn_boot import _ntff_profile_via_ctypes
        mod = types.ModuleType("antenv.axon_hooks")
        mod._hook = None
        mod.set_axon_ntff_profile_hook = lambda h: setattr(mod, "_hook", h)
        mod.get_axon_ntff_profile_hook = lambda: mod._hook
        sys.modules["antenv.axon_hooks"] = mod
        antenv.axon_hooks = mod
        mod.set_axon_ntff_profile_hook(_ntff_profile_via_ctypes('/opt/axon/libaxon_pjrt.so'))
    except Exception:
        pass


_NC_CACHE = {}


def build_program():
    if "nc" in _NC_CACHE:
        return _NC_CACHE["nc"]
    import concourse.bass as bass
    import concourse.mybir as mybir
    import concourse.tile as tile
    from concourse import bacc
    from concourse.masks import make_identity

    F32, BF16 = mybir.dt.float32, mybir.dt.bfloat16
    AF = mybir.ActivationFunctionType
    SCALE = 0.125
    G = 160
    VS_P = 8 * 6 * G

    nc = bacc.Bacc("TRN2", target_bir_lowering=False, debug=False)
    xt_d = [nc.dram_tensor(f"xt{b}", [768, 1025], BF16, kind="ExternalInput") for b in range(2)]
    wq_d = nc.dram_tensor("wqkv", [768, 2304], BF16, kind="ExternalInput")
    wp_d = nc.dram_tensor("wproj", [768, 768], BF16, kind="ExternalInput")
    bias_d = nc.dram_tensor("bias", [1, 768], BF16, kind="ExternalInput")
    tpl_d = nc.dram_tensor("tpl", [128, 160], BF16, kind="ExternalInput")
    ones_d = nc.dram_tensor("ones", [1, 128], BF16, kind="ExternalInput")
    out_d = [nc.dram_tensor(f"out{b}", [1025, 768], F32, kind="ExternalOutput") for b in range(2)]

    with tile.TileContext(nc) as tc:
        with (
            tc.tile_pool(name="big", bufs=1) as big,
            tc.tile_pool(name="pb", bufs=2) as pb,       # per-batch ring tiles
            tc.tile_pool(name="es", bufs=2) as es_pool,
            tc.tile_pool(name="ra", bufs=2) as ra_pool,
            tc.tile_pool(name="rb", bufs=2) as rb_pool,
            tc.tile_pool(name="st", bufs=2) as st_pool,
            tc.tile_pool(name="psA", bufs=2, space="PSUM") as psA,
            tc.tile_pool(name="psS", bufs=1, space="PSUM") as psS,
            tc.tile_pool(name="psO", bufs=2, space="PSUM") as psO,
            tc.tile_pool(name="psB", bufs=2, space="PSUM") as psB,
        ):
            # ---- persistent (loaded once, shared by both batches) ----
            wq_s = big.tile([128, 6, 2304], BF16, tag="wq")
            for k in range(6):
                nc.sync.dma_start(wq_s[:, k], wq_d.ap()[k * 128:(k + 1) * 128, :])
            wp_s = big.tile([128, 6, 768], BF16, tag="wp")
            for k in range(6):
                nc.sync.dma_start(wp_s[:, k], wp_d.ap()[k * 128:(k + 1) * 128, :])
            identf = big.tile([128, 128], F32, tag="identf")
            make_identity(nc, identf[:])
            # bias as a full-K matmul operand: row 0 = bias, rows 1:128 = 0
            biasz = big.tile([128, 768], BF16, tag="biasz")
            nc.sync.dma_start(biasz[0:1, :], bias_d.ap())
            for c in range(12):
                nc.sync.dma_start(biasz[1:128, c * 64:(c + 1) * 64], tpl_d.ap()[1:128, 0:64])
            onesz = big.tile([128, 128], BF16, tag="onesz")
            nc.sync.dma_start(onesz[0:1, :], ones_d.ap())
            for c in range(2):
                nc.sync.dma_start(onesz[1:128, c * 64:(c + 1) * 64], tpl_d.ap()[1:128, 0:64])

            def run_batch(b):
                xt = pb.tile([128, 6, 1025], BF16, tag="xt")
                qkt = pb.tile([128, 12, 1024], BF16, tag="qkt")
                vstore = pb.tile([128, 8, 6, G], BF16, tag="vs")
                at = xt  # aliased: every xt read happens in stage A before the first at write (B/D)
                x0f = pb.tile([128, 6], F32, tag="x0f")
                q0blk = pb.tile([128, 6, 12], BF16, tag="q0blk")
                k0cp = pb.tile([128, 6], BF16, tag="k0cp")
                kct2 = pb.tile([128, 12], BF16, tag="kct2")   # block-diag cls-key cols
                row0 = pb.tile([1, 2304], F32, tag="row0")  # qkv row of orig cls; cols 768:2304 reused in stage C
                v0a = pb.tile([128, 6, G], BF16, tag="v0a")   # row 0 = v0 groups, rows 1:128 template
                vca = pb.tile([128, 6, G], BF16, tag="vca")
                w_cls = pb.tile([12, 1025], F32, tag="wcls")
                wt_cls = pb.tile([128, 9, 12], BF16, tag="wtcls")
                part = pb.tile([12, 4], F32, tag="part")
                den_c = pb.tile([12, 2], F32, tag="denc")
                esc2a = pb.tile([128, 512], BF16, tag="esc0", name="esc2a")
                esc2b = pb.tile([128, 512], BF16, tag="esc1", name="esc2b")
                esc2 = [esc2a, esc2b]

                # ---- stage A ----
                for k in range(6):
                    nc.sync.dma_start(xt[:, k], xt_d[b].ap()[k * 128:(k + 1) * 128, :])
                nc.vector.tensor_copy(x0f[:], xt[:, :, 0])
                # augmented-V templates: zeros with ones at cols 64:96 of each group
                for t in range(8):
                    for g in range(6):
                        nc.sync.dma_start(vstore[:, t, g], tpl_d.ap())
                for g in range(6):
                    nc.sync.dma_start(v0a[:, g], tpl_d.ap())
                    nc.sync.dma_start(vca[:, g], tpl_d.ap())
                # esc2 rows 1:128 must be REAL zeros (they multiply template ones)
                for i in range(2):
                    for c in range(8):
                        nc.sync.dma_start(esc2[i][1:128, c * 64:(c + 1) * 64], tpl_d.ap()[1:128, 0:64])
                # q0blk zeros; wt_cls chunk-0 rows 1:128 zeros; kct2 zeros
                q0f = q0blk.rearrange("p a c -> p (a c)")
                nc.sync.dma_start(q0f[:, 0:64], tpl_d.ap()[:, 0:64])
                nc.sync.dma_start(q0f[:, 64:72], tpl_d.ap()[:, 96:104])
                nc.sync.dma_start(wt_cls[1:128, 0, :], tpl_d.ap()[1:128, 0:12])
                nc.sync.dma_start(kct2[:, :], tpl_d.ap()[:, 0:12])

                # full qkv row of the original cls token: row0 = x0^T W_qkv
                for (c0, cw) in ((0, 512), (512, 512), (1024, 512), (1536, 512), (2048, 256)):
                    r_ps = psB.tile([128, 512], F32, tag="cls")
                    for k in range(6):
                        nc.tensor.matmul(r_ps[0:1, 0:cw], xt[:, k, 0:1], wq_s[:, k, c0:c0 + cw],
                                         start=(k == 0), stop=(k == 5))
                    nc.vector.tensor_copy(row0[0:1, c0:c0 + cw], r_ps[0:1, 0:cw])
                # v0 augmented row (row 0 of v0a) + q0 block-diagonal lhsT + k0 column
                dstv0 = bass.AP(v0a.tensor, v0a.offset, [[6 * G, 1], [G, 4], [96, 2], [1, 64]])
                nc.vector.tensor_copy(dstv0, row0[0:1, 1536:2048].rearrange("p (g h d) -> p g h d", h=2, d=64))
                dstv1 = bass.AP(v0a.tensor, v0a.offset + 4 * G, [[6 * G, 1], [G, 2], [96, 2], [1, 64]])
                nc.vector.tensor_copy(dstv1, row0[0:1, 2048:2304].rearrange("p (g h d) -> p g h d", h=2, d=64))
                for k in range(6):
                    tq = psB.tile([128, 512], F32, tag="cls")
                    nc.tensor.transpose(tq[0:128, 0:1], row0[0:1, k * 128:(k + 1) * 128], identf[0:1, 0:1])
                    nc.vector.tensor_copy(q0blk[0:64, k, 2 * k:2 * k + 1], tq[0:64, 0:1])
                    nc.vector.tensor_copy(q0blk[64:128, k, 2 * k + 1:2 * k + 2], tq[64:128, 0:1])
                    tk = psB.tile([128, 512], F32, tag="cls")
                    nc.tensor.transpose(tk[0:128, 0:1], row0[0:1, 768 + k * 128:768 + (k + 1) * 128],
                                        identf[0:1, 0:1])
                    nc.vector.tensor_copy(k0cp[:, k:k + 1], tk[0:128, 0:1])

                # K features first (cls attention needs them), then V, then Q
                for m in list(range(6, 12)) + list(range(6)):
                    for ci in range(2):
                        qk_ps = psA.tile([128, 512], F32, tag="mm")
                        for k in range(6):
                            nc.tensor.matmul(qk_ps[:], wq_s[:, k, m * 128:(m + 1) * 128],
                                             xt[:, k, 1 + ci * 512: 1 + (ci + 1) * 512],
                                             start=(k == 0), stop=(k == 5))
                        nc.scalar.copy(qkt[:, m, ci * 512:(ci + 1) * 512], qk_ps[:])
                for t in range(8):
                    for (c0, cw, g0) in ((0, 512, 0), (512, 256, 4)):
                        v_ps = psA.tile([128, 512], F32, tag="mm")
                        for k in range(6):
                            nc.tensor.matmul(v_ps[:, 0:cw], xt[:, k, 1 + t * 128: 1 + (t + 1) * 128],
                                             wq_s[:, k, 1536 + c0:1536 + c0 + cw], start=(k == 0), stop=(k == 5))
                        dst = bass.AP(vstore.tensor, vstore.offset + t * 6 * G + g0 * G,
                                      [[VS_P, 128], [G, cw // 128], [96, 2], [1, 64]])
                        nc.vector.tensor_copy(dst, v_ps[:, 0:cw].rearrange("p (g h d) -> p g h d", h=2, d=64))

                # ---- stage B: cls attention ----
                for ci in range(2):
                    cl_ps = psB.tile([128, 512], F32, tag="cls")
                    for k in range(6):
                        nc.tensor.matmul(cl_ps[0:12, :], q0blk[:, k], qkt[:, 6 + k, ci * 512:(ci + 1) * 512],
                                         start=(k == 0), stop=(k == 5))
                    nc.scalar.activation(w_cls[:, 1 + ci * 512: 1 + (ci + 1) * 512], cl_ps[0:12, :],
                                         AF.Exp, scale=SCALE, accum_out=part[:, ci:ci + 1])
                cl0 = psB.tile([128, 512], F32, tag="cls")
                for k in range(6):
                    nc.tensor.matmul(cl0[0:12, 0:1], q0blk[:, k], k0cp[:, k:k + 1],
                                     start=(k == 0), stop=(k == 5))
                nc.scalar.activation(w_cls[:, 0:1], cl0[0:12, 0:1], AF.Exp, scale=SCALE,
                                     accum_out=part[:, 2:3])
                nc.vector.tensor_add(den_c[:, 0:1], part[:, 0:1], part[:, 1:2])
                nc.vector.tensor_add(den_c[:, 0:1], den_c[:, 0:1], part[:, 2:3])
                nc.vector.reciprocal(den_c[:, 1:2], den_c[:, 0:1])
                nc.vector.tensor_scalar_mul(w_cls[:], w_cls[:], den_c[:, 1:2])
                t0 = psB.tile([128, 512], F32, tag="cls")
                nc.tensor.transpose(t0[0:1, 0:12], w_cls[:, 0:1], identf[0:12, 0:12])
                nc.vector.tensor_copy(wt_cls[0:1, 0, :], t0[0:1, 0:12])
                for t in range(8):
                    tw = psB.tile([128, 512], F32, tag="cls")
                    nc.tensor.transpose(tw[0:128, 0:12], w_cls[:, 1 + t * 128: 1 + (t + 1) * 128],
                                        identf[0:12, 0:12])
                    nc.vector.tensor_copy(wt_cls[:, 1 + t, :], tw[0:128, 0:12])
                for fi in range(6):
                    co = psB.tile([128, 512], F32, tag="cls")
                    la0 = bass.AP(v0a.tensor, v0a.offset + fi * G, [[6 * G, 128], [1, 64]])
                    nc.tensor.matmul(co[0:64, 0:2], la0, wt_cls[:, 0, 2 * fi:2 * fi + 2],
                                     start=True, stop=False)
                    for t in range(8):
                        la = bass.AP(vstore.tensor, vstore.offset + t * 6 * G + fi * G,
                                     [[VS_P, 128], [1, 64]])
                        nc.tensor.matmul(co[0:64, 0:2], la, wt_cls[:, 1 + t, 2 * fi:2 * fi + 2],
                                         start=False, stop=(t == 7))
                    lb0 = bass.AP(v0a.tensor, v0a.offset + fi * G + 32, [[6 * G, 128], [1, 128]])
                    nc.tensor.matmul(co[0:128, 2:4], lb0, wt_cls[:, 0, 2 * fi:2 * fi + 2],
                                     start=True, stop=False)
                    for t in range(8):
                        lb = bass.AP(vstore.tensor, vstore.offset + t * 6 * G + fi * G + 32,
                                     [[VS_P, 128], [1, 128]])
                        nc.tensor.matmul(co[0:128, 2:4], lb, wt_cls[:, 1 + t, 2 * fi:2 * fi + 2],
                                         start=False, stop=(t == 7))
                    nc.vector.tensor_add(at[0:64, fi, 0:1], co[0:64, 0:1], x0f[0:64, fi:fi + 1])
                    nc.vector.tensor_add(at[64:128, fi, 0:1], co[64:128, 3:4], x0f[64:128, fi:fi + 1])

                # ---- stage C: updated cls k, v ----
                for (c0, cw) in ((768, 512), (1280, 512), (1792, 512)):
                    kv_ps = psB.tile([128, 512], F32, tag="cls")
                    for k in range(6):
                        nc.tensor.matmul(kv_ps[0:1, 0:cw], at[:, k, 0:1], wq_s[:, k, c0:c0 + cw],
                                         start=(k == 0), stop=(k == 5))
                    nc.vector.tensor_copy(row0[0:1, c0:c0 + cw], kv_ps[0:1, 0:cw])
                for k in range(6):
                    tk = psB.tile([128, 512], F32, tag="cls")
                    nc.tensor.transpose(tk[0:128, 0:1], row0[0:1, 768 + k * 128:768 + (k + 1) * 128],
                                        identf[0:1, 0:1])
                    nc.vector.tensor_copy(kct2[0:64, 2 * k:2 * k + 1], tk[0:64, 0:1])
                    nc.vector.tensor_copy(kct2[64:128, 2 * k + 1:2 * k + 2], tk[64:128, 0:1])
                dstc0 = bass.AP(vca.tensor, vca.offset, [[6 * G, 1], [G, 4], [96, 2], [1, 64]])
                nc.vector.tensor_copy(dstc0, row0[0:1, 1536:2048].rearrange("p (g h d) -> p g h d", h=2, d=64))
                dstc1 = bass.AP(vca.tensor, vca.offset + 4 * G, [[6 * G, 1], [G, 2], [96, 2], [1, 64]])
                nc.vector.tensor_copy(dstc1, row0[0:1, 2048:2304].rearrange("p (g h d) -> p g h d", h=2, d=64))

                # ---- stage D: branch attention ----
                for br in range(4):
                    for fi in range(6):
                        gi = br * 6 + fi
                        qsl = slice(br * 256, (br + 1) * 256)
                        es = es_pool.tile([128, 1024], BF16, tag="es")  # a0|b0|a1|b1
                        for half in range(2):
                            ksl = slice(br * 256 + half * 128, br * 256 + (half + 1) * 128)
                            # a and b run concurrently (different row groups) -> separate banks
                            ps_s = psS.tile([128, 1024], F32, tag="s")
                            nc.tensor.matmul(ps_s[:, 0:256], qkt[0:64, 6 + fi, ksl], qkt[0:64, fi, qsl],
                                             start=True, stop=True)
                            nc.tensor.matmul(ps_s[:, 512:768], qkt[64:128, 6 + fi, ksl], qkt[64:128, fi, qsl],
                                             start=True, stop=True)
                            src = bass.AP(ps_s.tensor, ps_s.offset, [[1024, 128], [512, 2], [1, 256]])
                            dst = es[:, half * 512:(half + 1) * 512].rearrange("p (c q) -> p c q", c=2)
                            nc.scalar.activation(dst, src, AF.Exp, scale=SCALE)
                        ps_c = psB.tile([128, 512], F32, tag="cls")
                        nc.tensor.matmul(ps_c[0:1, 0:256], kct2[:, 2 * fi:2 * fi + 1], qkt[:, fi, qsl],
                                         start=True, stop=True)
                        nc.tensor.matmul(ps_c[0:1, 256:512], kct2[:, 2 * fi + 1:2 * fi + 2], qkt[:, fi, qsl],
                                         start=True, stop=True)
                        ec = esc2[gi % 2]
                        nc.scalar.activation(ec[0:1, :], ps_c[0:1, :], AF.Exp, scale=SCALE)
                        ps_o = psO.tile([128, 512], F32, tag="o")  # a at cols 0:256, b at 256:512
                        lA0 = bass.AP(vca.tensor, vca.offset + fi * G, [[6 * G, 128], [1, 65]])
                        nc.tensor.matmul(ps_o[0:65, 0:256], lA0, ec[:, 0:256], start=True, stop=False)
                        for half in range(2):
                            tt = 2 * br + half
                            lA = bass.AP(vstore.tensor, vstore.offset + tt * 6 * G + fi * G,
                                         [[VS_P, 128], [1, 65]])
                            nc.tensor.matmul(ps_o[0:65, 0:256], lA, es[:, half * 512: half * 512 + 256],
                                             start=False, stop=(half == 1))
                        lB0 = bass.AP(vca.tensor, vca.offset + fi * G + 32, [[6 * G, 128], [1, 128]])
                        nc.tensor.matmul(ps_o[0:128, 256:512], lB0, ec[:, 256:512], start=True, stop=False)
                        for half in range(2):
                            tt = 2 * br + half
                            lB = bass.AP(vstore.tensor, vstore.offset + tt * 6 * G + fi * G + 32,
                                         [[VS_P, 128], [1, 128]])
                            nc.tensor.matmul(ps_o[0:128, 256:512], lB, es[:, half * 512 + 256:(half + 1) * 512],
                                             start=False, stop=(half == 1))
                        # reciprocal of denominators straight from PSUM, then partition-broadcast
                        ra = ra_pool.tile([128, 256], F32, tag="ra")
                        nc.vector.reciprocal(ra[64:65, :], ps_o[64:65, 0:256])
                        nc.vector.reciprocal(ra[32:33, :], ps_o[32:33, 256:512])
                        rb = rb_pool.tile([128, 256], F32, tag="rb")
                        nc.sync.dma_start(rb[0:64, :], bass.AP(ra.tensor, ra.offset + 64 * 256,
                                                               [[256, 1], [0, 64], [1, 256]]))
                        nc.sync.dma_start(rb[64:128, :], bass.AP(ra.tensor, ra.offset + 32 * 256,
                                                                 [[256, 1], [0, 64], [1, 256]]))
                        csl = slice(1 + br * 256, 1 + (br + 1) * 256)
                        nc.vector.tensor_mul(at[0:64, fi, csl], ps_o[0:64, 0:256], rb[0:64, :])
                        nc.vector.tensor_mul(at[64:128, fi, csl], ps_o[64:128, 256:512], rb[64:128, :])

                # ---- stage E: projection ----
                for (c0, cw) in ((0, 512), (512, 256)):
                    for mt in range(9):
                        m0, mw = (mt * 128, 128) if mt < 8 else (1024, 1)
                        pr = psA.tile([128, 512], F32, tag="mm")
                        for k in range(6):
                            nc.tensor.matmul(pr[0:mw, 0:cw], at[:, k, m0:m0 + mw], wp_s[:, k, c0:c0 + cw],
                                             start=(k == 0), stop=False)
                        nc.tensor.matmul(pr[0:mw, 0:cw], onesz[:, 0:mw], biasz[:, c0:c0 + cw],
                                         start=False, stop=True)
                        stg = st_pool.tile([128, 512], F32, tag="stg")
                        nc.vector.tensor_copy(stg[0:mw, 0:cw], pr[0:mw, 0:cw])
                        nc.sync.dma_start(out_d[b].ap()[m0:m0 + mw, c0:c0 + cw], stg[0:mw, 0:cw])

            run_batch(0)
            run_batch(1)

    nc.compile()
    _NC_CACHE["nc"] = nc
    return nc


def prep_inputs(x, W_qkv, W_proj, b_proj):
    """Host-side prep: transpose x and cast matmul operands to bf16."""
    import ml_dtypes
    bf16 = ml_dtypes.bfloat16
    x = np.asarray(x, dtype=np.float32)
    xt = np.ascontiguousarray(np.transpose(x, (0, 2, 1))).astype(bf16)
    wq = np.asarray(W_qkv, dtype=np.float32).astype(bf16)
    wp = np.asarray(W_proj, dtype=np.float32).astype(bf16)
    bias = np.ascontiguousarray(np.asarray(b_proj, dtype=np.float32).reshape(1, 768)).astype(bf16)
    tpl = np.zeros((128, 160), np.float32)
    tpl[:, 64:96] = 1.0
    tpl = tpl.astype(bf16)
    ones = np.ones((1, 128), np.float32).astype(bf16)
    return [{"xt0": xt[2 * c], "xt1": xt[2 * c + 1], "wqkv": wq, "wproj": wp, "bias": bias,
             "tpl": tpl, "ones": ones}
            for c in range(8)]


def kernel(x, W_qkv, W_proj, b_proj):
    _ensure_ntff_hook()
    from concourse import bass_utils
    nc = build_program()
    in_maps = prep_inputs(x, W_qkv, W_proj, b_proj)
    res = bass_utils.run_bass_kernel_spmd(nc, in_maps, list(range(8)))
    out = np.empty((16, 1025, 768), np.float32)
    for c in range(8):
        out[2 * c] = res.results[c]["out0"]
        out[2 * c + 1] = res.results[c]["out1"]
    return out
